# revision 24
# baseline (speedup 1.0000x reference)
"""DeepSeekV3-style MoE block on 8 Trainium2 NeuronCores.

Strategy (expert-parallel, host-routed dispatch/combine):
  - Host computes the (tiny) sigmoid gate in fp32 numpy, does top-2 selection
    and builds per-expert token lists (the "all-to-all dispatch" happens while
    sharding the inputs).
  - Core e runs expert e's SwiGLU over its gathered tokens (padded to a fixed
    capacity) plus a 1/8 token-slice of the shared expert.  Gate scaling is
    applied on-chip.  The host scatter-adds the per-core outputs back into the
    full [B,S,H] tensor (the "combine" happens while unsharding).

Numerics/performance: the up/gate projections (phase A) run in fp8(e4m3)
DoubleRow mode (2 contraction k-tiles per instruction at 0.5 PE cycles/row =
4x bf16 throughput) with 3-pass residual compensation:
    w@x  ~=  w_hi@x_hi + w_lo@x_hi + w_hi@x_lo
where v = v_hi + v_lo is an exact-ish two-term fp8 decomposition (residual
quantization), giving ~bf16-level accuracy at 0.75x the bf16 PE cost.  All
quantization scales are powers of two (x*2^5, w*2^10), so PSUM holds h*2^15;
the SiLU activation applies 2^-15 as its input scale and the 2^-15 on the h3
factor is folded into the host-side pre-scaling of w2 (exact in bf16).
Phase B (down-proj) stays bf16: act_sb = silu(h1) * (h3*2^15), y = act_sb @
(w2*2^-15).

All matmuls are laid out so no on-chip transposes are needed:
  phase A:  act[f,c] = silu(w1[h,f].T @ x[h,c]) * (w3[h,f].T @ x[h,c])
  phase B:  y[c,h]   = act[f,c].T @ w2[f,h]    (scaled by the gate weight)
Host-side pre-tiling puts every DRAM operand in [128, ...] partition-major
layout so each DMA is contiguous.
"""

import hashlib
import os
import sys

for _p in ("/opt/trn_rl_repo", "/opt/pypackages"):
    if _p not in sys.path:
        sys.path.append(_p)

from contextlib import ExitStack

import numpy as np
import ml_dtypes

import concourse.bacc as bacc
import concourse.mybir as mybir
import concourse.tile as tile
from concourse import bass2jax
from concourse.bass_utils import run_bass_kernel_spmd

_NEFF_CACHE_DIR = os.path.expanduser("~/.cache/bass_neff_cache")
_active_build_key = None   # set by _get_nc around the PJRT dispatch


def _install_neff_cache():
    """Persist the compiled bass_exec NEFF across processes.

    The walrus backend takes minutes for this kernel and has no cache of its
    own.  The HLO bytes are not byte-stable across processes (volatile ids /
    debug metadata), so the cache key is derived from the *build inputs*
    (capacities + CFG + build source) instead.  Only the renamed NEFF bytes
    are stored; each request re-wraps them around its own HLO."""
    if getattr(bass2jax, "_ant_neff_cache_wrapped", False):
        return
    inner = bass2jax.neuronx_cc_hook

    captured = {}
    orig_rename = bass2jax.rename_neff_tensors_and_patch_header

    def capture_rename(neff_path, mapping):
        data = orig_rename(neff_path, mapping)
        captured["neff"] = data
        return data

    bass2jax.rename_neff_tensors_and_patch_header = capture_rename

    def cached_hook(code, code_format, platform_version, file_prefix):
        c = code if isinstance(code, (bytes, bytearray)) else str(code).encode()
        if b"bass_exec" not in c or _active_build_key is None:
            return inner(code, code_format, platform_version, file_prefix)
        from libneuronxla.libncc import _wrap_neff_as_custom_call

        path = os.path.join(_NEFF_CACHE_DIR, _active_build_key + ".neff")
        try:
            if os.path.exists(path):
                with open(path, "rb") as f:
                    return 0, _wrap_neff_as_custom_call(bytes(c), f.read())
        except Exception:
            pass
        captured.pop("neff", None)
        r = inner(code, code_format, platform_version, file_prefix)
        neff = captured.pop("neff", None)
        if neff is not None:
            try:
                os.makedirs(_NEFF_CACHE_DIR, exist_ok=True)
                tmp = f"{path}.tmp{os.getpid()}"
                with open(tmp, "wb") as f:
                    f.write(neff)
                os.replace(tmp, path)
            except Exception:
                pass
        return r

    bass2jax.neuronx_cc_hook = cached_hook
    bass2jax._ant_neff_cache_wrapped = True


_install_neff_cache()


def _build_key(C_r, C_s):
    import inspect

    src = inspect.getsource(_build) + inspect.getsource(_chunks)
    blob = f"moe-ep-v6|{C_r}|{C_s}|{SX}|{SW}|{SA}|{SW2}|{sorted(CFG.items())}|{src}"
    return hashlib.sha256(blob.encode()).hexdigest()

F8 = ml_dtypes.float8_e4m3   # device float8e4 is IEEE e4m3: max 240, has inf
BF16 = ml_dtypes.bfloat16
P = 128
H = 2048
F = 1408
E = 8
TOPK = 2
NCORES = 8
KH = H // P   # 16 contraction tiles over H
KF = F // P   # 11 contraction tiles over F
HB = H // 512  # 4 output column blocks

SX = 2.0 ** 5    # x quant scale (|x| < 7 -> |x*32| < 240)
SW = 2.0 ** 10   # w1/w3 quant scale (|w| < 0.23)
DEQ = 1.0 / (SX * SW)   # PSUM -> h scale
SA = 2.0 ** 3    # act quant scale (|act| < 30 -> < 240)
SW2 = 2.0 ** 10  # w2 quant scale
DEQ2 = 1.0 / (SA * SW2)  # phase-B PSUM -> y scale
KFP = KF + 1     # f-tiles padded to an even count for DoubleRow pairing

FP32 = mybir.dt.float32
BF16_DT = mybir.dt.bfloat16
F8_DT = mybir.dt.float8e4
DR = mybir.MatmulPerfMode.DoubleRow


def _chunks(C, first=None):
    """Split C into 512-wide chunks (+ remainder).  `first` optionally
    shrinks the leading chunk so the kernel's first matmuls wait on a smaller
    x transfer."""
    out = []
    c0 = 0
    if first and first < C:
        out.append((0, first))
        c0 = first
    while c0 < C:
        cb = min(512, C - c0)
        out.append((c0, cb))
        c0 += cb
    return out


CFG = {
    # Every dma_start costs ~625ns on the (global) HWDGE queue in addition to
    # its transfer time, so the kernel uses few, large DMAs.
    "w13_bufs": 4,    # per-f [w1h|w1l|w3h|w3l] tile ring depth
    "x_chunk0": 384,  # columns in the startup-critical first x piece
    "w2_defer_f": 4,  # emit the w2 bulk load at this f iteration
    "w2_split": 2,    # dma_starts for the w2 bulk load
    "ps1_bufs": 2,
    "ps2_bufs": 3,
    "o_bufs": 3,
    "silu_bufs": 3,
    "out_bf16": True,   # store outputs as bf16 (halves output DMA)
    "warmup_mms": 20,   # dummy matmuls at t=0: warm the PE clock (HAM) while
                        # the first real DMAs are still in flight
}


def _split_dma(eng, dst, src, n):
    w = dst.shape[-1]
    step = -(-w // n)
    for i in range(0, w, step):
        j = min(w, i + step)
        eng.dma_start(dst[:, i:j], src[:, i:j])


def _build(nc, C_r, C_s):
    """Emit the per-core program: routed expert (C_r tokens, gated) then the
    shared-expert slice (C_s tokens)."""
    dram = {}
    for name, shape, dt in [
        ("xrh", [P, KH * C_r], F8_DT),
        ("xrl", [P, KH * C_r], F8_DT),
        ("gr", [P, -(-C_r // P)], FP32),
        # per-f blocks of [w1h | w1l | w3h | w3l], each KH*P columns
        ("w13", [P, KF * 4 * KH * P], F8_DT),
        ("w2h", [P, KFP * H], F8_DT),
        ("w2l", [P, KFP * H], F8_DT),
        ("xsh", [P, KH * C_s], F8_DT),
        ("xsl", [P, KH * C_s], F8_DT),
        ("s13", [P, KF * 4 * KH * P], F8_DT),
        ("s2h", [P, KFP * H], F8_DT),
        ("s2l", [P, KFP * H], F8_DT),
    ]:
        dram[name] = nc.dram_tensor(name, shape, dt, kind="ExternalInput")
    out_dt = BF16_DT if CFG["out_bf16"] else FP32
    yr = nc.dram_tensor("yr", [C_r, H], out_dt, kind="ExternalOutput")
    ys = nc.dram_tensor("ys", [C_s, H], out_dt, kind="ExternalOutput")

    with tile.TileContext(nc) as tc, ExitStack() as ctx:
        pool = ctx.enter_context(tc.tile_pool(name="main", bufs=1))
        psum = ctx.enter_context(tc.tile_pool(name="ps", bufs=1, space="PSUM"))

        if CFG["warmup_mms"]:
            # No DMA dependency: memset SBUF, then back-to-back matmuls so the
            # PE HAM/p-state is warm by the time the first weights arrive.
            wz = pool.tile([P, P], BF16_DT, tag="warm_w", bufs=1)
            rz = pool.tile([P, 512], BF16_DT, tag="warm_r", bufs=1)
            nc.gpsimd.memset(wz[:], 0.0)
            nc.gpsimd.memset(rz[:], 0.0)
            pz = psum.tile([P, 512], FP32, tag="warm_ps", bufs=1)
            for _ in range(CFG["warmup_mms"]):
                nc.tensor.matmul(pz[:], lhsT=wz[:], rhs=rz[:], start=True,
                                 stop=True)

        def problem(tag, xhd, xld, w13d, w2hd, w2ld, yd, C, gd=None):
            # resident x (fp8 hi/lo): [128, KH, C]
            xh_sb = pool.tile([P, KH, C], F8_DT, tag=f"xh_{tag}", bufs=1)
            xl_sb = pool.tile([P, KH, C], F8_DT, tag=f"xl_{tag}", bufs=1)
            g_sb = None
            if gd is not None:
                g_sb = pool.tile([P, -(-C // P)], FP32, tag=f"g_{tag}", bufs=1)
                nc.sync.dma_start(g_sb[:], gd[:])

            w2h_sb = pool.tile([P, KFP, H], F8_DT, tag="w2h", bufs=1)
            w2l_sb = pool.tile([P, KFP, H], F8_DT, tag="w2l", bufs=1)
            ah_sb = pool.tile([P, KFP, C], F8_DT, tag=f"ah_{tag}", bufs=1)
            al_sb = pool.tile([P, KFP, C], F8_DT, tag=f"al_{tag}", bufs=1)
            # zero f-padding tile so the DoubleRow pairs contract over KFP
            nc.gpsimd.memset(ah_sb[:, KF, :], 0.0)
            nc.gpsimd.memset(al_sb[:, KF, :], 0.0)

            # x loads: two strided pieces per tensor (chunk0 first so phase A
            # can start, then the remainder), hi before lo (the x_lo passes
            # come last in each accumulation group).  Emitted inside the
            # f-loop so the startup-critical w13 f=0 tile wins the DMA queue.
            cb0 = min(CFG["x_chunk0"], C)
            xpieces = [[(xh_sb, xhd, 0, cb0), (xl_sb, xld, 0, cb0)]]
            if cb0 < C:
                xpieces.append([(xh_sb, xhd, cb0, C - cb0),
                                (xl_sb, xld, cb0, C - cb0)])

            def emit_x(pieces):
                for xsb, xdr, c0, cw in pieces:
                    nc.sync.dma_start(
                        xsb[:, :, c0 : c0 + cw],
                        xdr.rearrange("p (k c) -> p k c", k=KH)[:, :, c0 : c0 + cw],
                    )

            wtiles = {}

            def load_w13(f):
                wt = pool.tile([P, 4, KH, P], F8_DT, tag="w13",
                               bufs=CFG["w13_bufs"])
                nc.sync.dma_start(
                    wt[:].rearrange("p t k j -> p (t k j)"),
                    w13d[:, f * 4 * KH * P : (f + 1) * 4 * KH * P],
                )
                wtiles[f] = wt

            # startup order: w13 f=0 first (first matmul operand), then the
            # x pieces interleaved with the next w13 tile.  All x DMAs must
            # be emitted before any matmul that reads them (the tile
            # framework tracks dependencies in emission order).
            load_w13(0)
            emit_x(xpieces[0])
            load_w13(1)
            for pieces in xpieces[1:]:
                emit_x(pieces)

            # ---- phase A: act[f, c] = silu(x@w1.T) * (x@w3.T), [F, C] layout
            # fp8 DoubleRow, 3 residual passes per psum accumulation group
            # (first compute chunk aligned with the first x piece)
            chunks = _chunks(C, first=CFG["x_chunk0"])
            for f in range(KF):
                if f not in wtiles:
                    load_w13(f)
                wf = wtiles.pop(f)
                if f == CFG["w2_defer_f"]:
                    # defer the (large, phase-B-only) w2 load past startup
                    _split_dma(nc.sync,
                               w2h_sb[:].rearrange("p k h -> p (k h)"),
                               w2hd[:], CFG["w2_split"])
                    _split_dma(nc.sync,
                               w2l_sb[:].rearrange("p k h -> p (k h)"),
                               w2ld[:], CFG["w2_split"])
                for ci, (c0, cb) in enumerate(chunks):
                    ps1 = psum.tile([P, cb], FP32, tag="ps1", bufs=CFG["ps1_bufs"])
                    ps3 = psum.tile([P, cb], FP32, tag="ps3", bufs=CFG["ps1_bufs"])
                    # interleaved psum groups: the four x_hi passes run first
                    # so the startup x_lo transfer has 32 instructions of slack
                    passes = [(ps1, 0, xh_sb, "s"), (ps1, 1, xh_sb, None),
                              (ps3, 2, xh_sb, "s"), (ps3, 3, xh_sb, None),
                              (ps1, 0, xl_sb, "e"), (ps3, 2, xl_sb, "e")]
                    for ps, t, xt, flag in passes:
                        for kk in range(0, KH, 2):
                            nc.tensor.matmul(
                                ps[:],
                                lhsT=wf[:, t, kk : kk + 2, :],
                                rhs=xt[:, kk : kk + 2, c0 : c0 + cb],
                                start=(flag == "s" and kk == 0),
                                stop=(flag == "e" and kk == KH - 2),
                                perf_mode=DR,
                            )
                    tmp = pool.tile([P, cb], BF16_DT, tag="silu", bufs=CFG["silu_bufs"])
                    nc.scalar.activation(
                        tmp[:], ps1[:], mybir.ActivationFunctionType.Silu,
                        scale=DEQ,
                    )
                    h3s = pool.tile([P, cb], BF16_DT, tag="h3s",
                                    bufs=CFG["silu_bufs"])
                    nc.scalar.activation(
                        h3s[:], ps3[:], mybir.ActivationFunctionType.Copy,
                        scale=DEQ * SA,
                    )
                    af = pool.tile([P, cb], BF16_DT, tag="actf",
                                   bufs=CFG["silu_bufs"])
                    nc.vector.tensor_mul(af[:], tmp[:], h3s[:])
                    ah = ah_sb[:, f, c0 : c0 + cb]
                    nc.gpsimd.tensor_copy(ah, af[:])
                    nc.vector.tensor_sub(
                        al_sb[:, f, c0 : c0 + cb], af[:], ah
                    )

            # ---- phase B: y[c, h] = act.T @ w2, fp8 DoubleRow 3-pass
            # (act carries 2^3, w2 carries 2^10; 2^-13 is folded into the
            # host gates / the shared-path epilogue scale)
            for ct in range(-(-C // P)):
                tp = min(P, C - ct * P)   # partial final token-tile
                o = pool.tile([P, HB * 512], out_dt, tag="o", bufs=CFG["o_bufs"])
                for hb in range(HB):
                    ps2 = psum.tile([P, 512], FP32, tag="ps2", bufs=CFG["ps2_bufs"])
                    n = KFP // 2 * 3
                    i = 0
                    for acts, w2s in ((ah_sb, w2h_sb), (al_sb, w2h_sb),
                                      (ah_sb, w2l_sb)):
                        for kk in range(0, KFP, 2):
                            nc.tensor.matmul(
                                ps2[:tp],
                                lhsT=acts[:, kk : kk + 2, ct * P : ct * P + tp],
                                rhs=w2s[:, kk : kk + 2,
                                        hb * 512 : (hb + 1) * 512],
                                start=(i == 0),
                                stop=(i == n - 1),
                                perf_mode=DR,
                            )
                            i += 1
                    if g_sb is not None:
                        nc.vector.tensor_scalar_mul(
                            o[:tp, hb * 512 : (hb + 1) * 512], ps2[:tp],
                            g_sb[:tp, ct : ct + 1]
                        )
                    else:
                        nc.vector.tensor_scalar_mul(
                            o[:tp, hb * 512 : (hb + 1) * 512], ps2[:tp], DEQ2
                        )
                        # shared problem is the kernel tail: store per-hb so
                        # the final DMA isn't gated on all four copies
                        nc.sync.dma_start(
                            yd[ct * P : ct * P + tp,
                               hb * 512 : (hb + 1) * 512],
                            o[:tp, hb * 512 : (hb + 1) * 512],
                        )
                if g_sb is not None:
                    # one store per token-tile covering all HB blocks
                    nc.sync.dma_start(yd[ct * P : ct * P + tp, :], o[:tp, :])

        d = {k: v.ap() for k, v in dram.items()}
        problem("r", d["xrh"], d["xrl"], d["w13"], d["w2h"], d["w2l"],
                yr.ap(), C_r, gd=d["gr"])
        problem("s", d["xsh"], d["xsl"], d["s13"], d["s2h"], d["s2l"],
                ys.ap(), C_s)

    return nc


_cache = {}


def _get_nc(C_r, C_s):
    key = (C_r, C_s, tuple(sorted(CFG.items())))
    if key not in _cache:
        nc = bacc.Bacc("TRN2", target_bir_lowering=False, debug=False,
                       num_devices=NCORES)
        _build(nc, C_r, C_s)
        nc.compile()
        _cache[key] = nc
    return _cache[key]


def _fp8_split(a, scale):
    """v -> (hi, lo) e4m3 pair with v*scale ~= hi + lo."""
    s = np.asarray(a, np.float32) * scale
    np.clip(s, -240.0, 240.0, out=s)
    hi = s.astype(F8)
    lo = s - hi.astype(np.float32)
    np.clip(lo, -240.0, 240.0, out=lo)
    return hi, lo.astype(F8)


def _tile_w13(w):
    """[F, H] -> [128, KF*KH*128], (f, kk, j) column order (any dtype)."""
    return np.ascontiguousarray(
        w.reshape(KF, P, KH, P).transpose(3, 0, 2, 1)
    ).reshape(P, KF * KH * P)


def _tile_w13_fp8(w):
    hi, lo = _fp8_split(w, SW)
    return _tile_w13(hi), _tile_w13(lo)


def _cat_w13(w1, w3):
    """[P, KF*4*KH*P] fp8 with per-f blocks [w1h | w1l | w3h | w3l]."""
    w1h, w1l = _tile_w13_fp8(w1)
    w3h, w3l = _tile_w13_fp8(w3)
    khp = KH * P
    parts = np.stack(
        [a.reshape(P, KF, khp) for a in (w1h, w1l, w3h, w3l)], axis=2
    )
    return np.ascontiguousarray(parts).reshape(P, KF * 4 * khp)


def _tile_w2_fp8(w):
    """[H, F] fp32 -> ([128, KFP*H] hi, lo) fp8, (f, h) order, zero-padded
    to KFP f-tiles."""
    hi, lo = _fp8_split(w, SW2)
    out = []
    for a in (hi, lo):
        t = np.zeros((P, KFP, H), F8)
        t[:, :KF, :] = np.ascontiguousarray(a).reshape(H, KF, P).transpose(2, 1, 0)
        out.append(t.reshape(P, KFP * H))
    return out


def _tile_x(x):
    """[C, H] -> [128, KH*C], (kk, c) column order (any dtype)."""
    C = x.shape[0]
    return np.ascontiguousarray(x.reshape(C, KH, P).transpose(2, 1, 0)).reshape(
        P, KH * C
    )


def _tile_x_fp8(x):
    hi, lo = _fp8_split(x, SX)
    return _tile_x(hi), _tile_x(lo)


def _pad_rows(x, n):
    if x.shape[0] == n:
        return x
    out = np.zeros((n, x.shape[1]), x.dtype)
    out[: x.shape[0]] = x
    return out


def kernel(hidden_states, gate_w, bias, ws1, ws2, ws3, we1, we2, we3):
    orig_shape = hidden_states.shape
    x = np.ascontiguousarray(
        np.asarray(hidden_states, np.float32).reshape(-1, orig_shape[-1])
    )
    T = x.shape[0]
    gate_w = np.asarray(gate_w, np.float32)
    bias = np.asarray(bias, np.float32)
    we1 = np.asarray(we1, np.float32)
    we2 = np.asarray(we2, np.float32)
    we3 = np.asarray(we3, np.float32)
    assert gate_w.shape[0] == E and we1.shape[0] == E and x.shape[1] == H

    # ---- host router (fp32, matches the reference's selection math)
    logits = x @ gate_w.T                                 # [T, E]
    scores = np.where(
        logits >= 0,
        1.0 / (1.0 + np.exp(-np.abs(logits))),
        1.0 - 1.0 / (1.0 + np.exp(-np.abs(logits))),
    ).astype(np.float32)
    routing = scores + bias[None, :]
    topk = np.argsort(-routing, axis=1, kind="stable")[:, :TOPK]  # [T, K]
    sel = np.take_along_axis(scores, topk, axis=1)
    gates = sel / sel.sum(axis=1, keepdims=True)          # [T, K]

    idx_e = []      # token ids routed to expert e
    gate_e = []     # matching combine weights
    for e in range(E):
        mask = topk == e                      # [T, K], at most one True per row
        rows = np.nonzero(mask.any(axis=1))[0]
        idx_e.append(rows)
        gate_e.append(gates[mask].astype(np.float32))  # row-major -> rows order

    max_n = max(len(r) for r in idx_e)
    # multiple of 16: DoubleRow stationary APs stride by C bytes and the
    # PE requires that stride to be 16-byte aligned
    C_r = max(64, -(-max_n // 16) * 16)
    C_s = max(64, -(-T // (NCORES * 64)) * 64)  # shared tokens per core

    nc = _get_nc(C_r, C_s)

    # ---- build per-core input maps
    s2h, s2l = _tile_w2_fp8(ws2)
    shared_w = {
        "s13": _cat_w13(ws1, ws3),
        "s2h": s2h, "s2l": s2l,
    }
    in_maps = []
    for e in range(E):
        rows = idx_e[e]
        xg = np.zeros((C_r, H), np.float32)
        xg[: len(rows)] = x[rows]
        ctiles = -(-C_r // P)
        g = np.zeros((ctiles * P,), np.float32)
        g[: len(rows)] = gate_e[e] * DEQ2
        xrh, xrl = _tile_x_fp8(xg)
        xsh, xsl = _tile_x_fp8(_pad_rows(x[e * C_s : (e + 1) * C_s], C_s))
        w2h, w2l = _tile_w2_fp8(we2[e])
        m = {
            "xrh": xrh, "xrl": xrl,
            "gr": np.ascontiguousarray(g.reshape(ctiles, P).T),
            "w13": _cat_w13(we1[e], we3[e]),
            "w2h": w2h, "w2l": w2l,
            "xsh": xsh, "xsl": xsl,
        }
        m.update(shared_w)
        in_maps.append(m)

    global _active_build_key
    _active_build_key = _build_key(C_r, C_s)
    try:
        res = run_bass_kernel_spmd(nc, in_maps, list(range(NCORES))).results
    finally:
        _active_build_key = None

    # ---- host combine
    out = np.zeros((T, H), np.float32)
    for e in range(E):
        rows = idx_e[e]
        out[rows] += np.asarray(res[e]["yr"][: len(rows)], np.float32)
        lo = e * C_s
        hi = min(T, (e + 1) * C_s)
        if lo < hi:
            out[lo:hi] += np.asarray(res[e]["ys"][: hi - lo], np.float32)
    return out.reshape(orig_shape).astype(np.float32)


# revision 25
# speedup vs baseline: 1.0132x; 1.0132x over previous
"""DeepSeekV3-style MoE block on 8 Trainium2 NeuronCores.

Strategy (expert-parallel, host-routed dispatch/combine):
  - Host computes the (tiny) sigmoid gate in fp32 numpy, does top-2 selection
    and builds per-expert token lists (the "all-to-all dispatch" happens while
    sharding the inputs).
  - Core e runs expert e's SwiGLU over its gathered tokens (padded to a fixed
    capacity) plus a 1/8 token-slice of the shared expert.  Gate scaling is
    applied on-chip.  The host scatter-adds the per-core outputs back into the
    full [B,S,H] tensor (the "combine" happens while unsharding).

Numerics/performance: the up/gate projections (phase A) run in fp8(e4m3)
DoubleRow mode (2 contraction k-tiles per instruction at 0.5 PE cycles/row =
4x bf16 throughput) with 3-pass residual compensation:
    w@x  ~=  w_hi@x_hi + w_lo@x_hi + w_hi@x_lo
where v = v_hi + v_lo is an exact-ish two-term fp8 decomposition (residual
quantization), giving ~bf16-level accuracy at 0.75x the bf16 PE cost.  All
quantization scales are powers of two (x*2^5, w*2^10), so PSUM holds h*2^15;
the SiLU activation applies 2^-15 as its input scale and the 2^-15 on the h3
factor is folded into the host-side pre-scaling of w2 (exact in bf16).
Phase B (down-proj) stays bf16: act_sb = silu(h1) * (h3*2^15), y = act_sb @
(w2*2^-15).

All matmuls are laid out so no on-chip transposes are needed:
  phase A:  act[f,c] = silu(w1[h,f].T @ x[h,c]) * (w3[h,f].T @ x[h,c])
  phase B:  y[c,h]   = act[f,c].T @ w2[f,h]    (scaled by the gate weight)
Host-side pre-tiling puts every DRAM operand in [128, ...] partition-major
layout so each DMA is contiguous.
"""

import hashlib
import os
import sys

for _p in ("/opt/trn_rl_repo", "/opt/pypackages"):
    if _p not in sys.path:
        sys.path.append(_p)

from contextlib import ExitStack

import numpy as np
import ml_dtypes

import concourse.bacc as bacc
import concourse.mybir as mybir
import concourse.tile as tile
from concourse import bass2jax
from concourse.bass_utils import run_bass_kernel_spmd

_NEFF_CACHE_DIR = os.path.expanduser("~/.cache/bass_neff_cache")
_active_build_key = None   # set by _get_nc around the PJRT dispatch


def _install_neff_cache():
    """Persist the compiled bass_exec NEFF across processes.

    The walrus backend takes minutes for this kernel and has no cache of its
    own.  The HLO bytes are not byte-stable across processes (volatile ids /
    debug metadata), so the cache key is derived from the *build inputs*
    (capacities + CFG + build source) instead.  Only the renamed NEFF bytes
    are stored; each request re-wraps them around its own HLO."""
    if getattr(bass2jax, "_ant_neff_cache_wrapped", False):
        return
    inner = bass2jax.neuronx_cc_hook

    captured = {}
    orig_rename = bass2jax.rename_neff_tensors_and_patch_header

    def capture_rename(neff_path, mapping):
        data = orig_rename(neff_path, mapping)
        captured["neff"] = data
        return data

    bass2jax.rename_neff_tensors_and_patch_header = capture_rename

    def cached_hook(code, code_format, platform_version, file_prefix):
        c = code if isinstance(code, (bytes, bytearray)) else str(code).encode()
        if b"bass_exec" not in c or _active_build_key is None:
            return inner(code, code_format, platform_version, file_prefix)
        from libneuronxla.libncc import _wrap_neff_as_custom_call

        path = os.path.join(_NEFF_CACHE_DIR, _active_build_key + ".neff")
        try:
            if os.path.exists(path):
                with open(path, "rb") as f:
                    return 0, _wrap_neff_as_custom_call(bytes(c), f.read())
        except Exception:
            pass
        captured.pop("neff", None)
        r = inner(code, code_format, platform_version, file_prefix)
        neff = captured.pop("neff", None)
        if neff is not None:
            try:
                os.makedirs(_NEFF_CACHE_DIR, exist_ok=True)
                tmp = f"{path}.tmp{os.getpid()}"
                with open(tmp, "wb") as f:
                    f.write(neff)
                os.replace(tmp, path)
            except Exception:
                pass
        return r

    bass2jax.neuronx_cc_hook = cached_hook
    bass2jax._ant_neff_cache_wrapped = True


_install_neff_cache()


def _build_key(C_r, C_s):
    import inspect

    src = inspect.getsource(_build) + inspect.getsource(_chunks)
    blob = f"moe-ep-v6|{C_r}|{C_s}|{SX}|{SW}|{SA}|{SW2}|{sorted(CFG.items())}|{src}"
    return hashlib.sha256(blob.encode()).hexdigest()

F8 = ml_dtypes.float8_e4m3   # device float8e4 is IEEE e4m3: max 240, has inf
BF16 = ml_dtypes.bfloat16
P = 128
H = 2048
F = 1408
E = 8
TOPK = 2
NCORES = 8
KH = H // P   # 16 contraction tiles over H
KF = F // P   # 11 contraction tiles over F
HB = H // 512  # 4 output column blocks

SX = 2.0 ** 5    # x quant scale (|x| < 7 -> |x*32| < 240)
SW = 2.0 ** 10   # w1/w3 quant scale (|w| < 0.23)
DEQ = 1.0 / (SX * SW)   # PSUM -> h scale
SA = 2.0 ** 3    # act quant scale (|act| < 30 -> < 240)
SW2 = 2.0 ** 10  # w2 quant scale
DEQ2 = 1.0 / (SA * SW2)  # phase-B PSUM -> y scale
KFP = KF + 1     # f-tiles padded to an even count for DoubleRow pairing

FP32 = mybir.dt.float32
BF16_DT = mybir.dt.bfloat16
F8_DT = mybir.dt.float8e4
DR = mybir.MatmulPerfMode.DoubleRow


def _chunks(C, first=None):
    """Split C into 512-wide chunks (+ remainder).  `first` optionally
    shrinks the leading chunk so the kernel's first matmuls wait on a smaller
    x transfer."""
    out = []
    c0 = 0
    if first and first < C:
        out.append((0, first))
        c0 = first
    while c0 < C:
        cb = min(512, C - c0)
        out.append((c0, cb))
        c0 += cb
    return out


CFG = {
    # Every dma_start costs ~625ns on the (global) HWDGE queue in addition to
    # its transfer time, so the kernel uses few, large DMAs.
    "w13_bufs": 4,    # per-f [w1h|w1l|w3h|w3l] tile ring depth
    "x_chunk0": 512,  # columns in the startup-critical first x piece
    "w2_defer_f": 4,  # emit the w2 bulk load at this f iteration
    "w2_split": 2,    # dma_starts for the w2 bulk load
    "ps1_bufs": 2,
    "ps2_bufs": 3,
    "o_bufs": 3,
    "silu_bufs": 3,
    "out_bf16": True,   # store outputs as bf16 (halves output DMA)
    "warmup_mms": 16,   # dummy matmuls at t=0: warm the PE clock (HAM) while
                        # the first real DMAs are still in flight
}


def _split_dma(eng, dst, src, n):
    w = dst.shape[-1]
    step = -(-w // n)
    for i in range(0, w, step):
        j = min(w, i + step)
        eng.dma_start(dst[:, i:j], src[:, i:j])


def _build(nc, C_r, C_s):
    """Emit the per-core program: routed expert (C_r tokens, gated) then the
    shared-expert slice (C_s tokens)."""
    dram = {}
    for name, shape, dt in [
        ("xrh", [P, KH * C_r], F8_DT),
        ("xrl", [P, KH * C_r], F8_DT),
        ("gr", [P, -(-C_r // P)], FP32),
        # per-f blocks of [w1h | w1l | w3h | w3l], each KH*P columns
        ("w13", [P, KF * 4 * KH * P], F8_DT),
        ("w2h", [P, KFP * H], F8_DT),
        ("w2l", [P, KFP * H], F8_DT),
        ("xsh", [P, KH * C_s], F8_DT),
        ("xsl", [P, KH * C_s], F8_DT),
        ("s13", [P, KF * 4 * KH * P], F8_DT),
        ("s2h", [P, KFP * H], F8_DT),
        ("s2l", [P, KFP * H], F8_DT),
    ]:
        dram[name] = nc.dram_tensor(name, shape, dt, kind="ExternalInput")
    out_dt = BF16_DT if CFG["out_bf16"] else FP32
    yr = nc.dram_tensor("yr", [C_r, H], out_dt, kind="ExternalOutput")
    ys = nc.dram_tensor("ys", [C_s, H], out_dt, kind="ExternalOutput")

    with tile.TileContext(nc) as tc, ExitStack() as ctx:
        pool = ctx.enter_context(tc.tile_pool(name="main", bufs=1))
        psum = ctx.enter_context(tc.tile_pool(name="ps", bufs=1, space="PSUM"))

        if CFG["warmup_mms"]:
            # No DMA dependency: memset SBUF, then back-to-back matmuls so the
            # PE HAM/p-state is warm by the time the first weights arrive.
            wz = pool.tile([P, P], BF16_DT, tag="warm_w", bufs=1)
            rz = pool.tile([P, 512], BF16_DT, tag="warm_r", bufs=1)
            nc.gpsimd.memset(wz[:], 0.0)
            nc.gpsimd.memset(rz[:], 0.0)
            pz = psum.tile([P, 512], FP32, tag="warm_ps", bufs=1)
            for _ in range(CFG["warmup_mms"]):
                nc.tensor.matmul(pz[:], lhsT=wz[:], rhs=rz[:], start=True,
                                 stop=True)

        def problem(tag, xhd, xld, w13d, w2hd, w2ld, yd, C, gd=None):
            # resident x (fp8 hi/lo): [128, KH, C]
            xh_sb = pool.tile([P, KH, C], F8_DT, tag=f"xh_{tag}", bufs=1)
            xl_sb = pool.tile([P, KH, C], F8_DT, tag=f"xl_{tag}", bufs=1)
            g_sb = None
            if gd is not None:
                g_sb = pool.tile([P, -(-C // P)], FP32, tag=f"g_{tag}", bufs=1)
                nc.sync.dma_start(g_sb[:], gd[:])

            w2h_sb = pool.tile([P, KFP, H], F8_DT, tag="w2h", bufs=1)
            w2l_sb = pool.tile([P, KFP, H], F8_DT, tag="w2l", bufs=1)
            ah_sb = pool.tile([P, KFP, C], F8_DT, tag=f"ah_{tag}", bufs=1)
            al_sb = pool.tile([P, KFP, C], F8_DT, tag=f"al_{tag}", bufs=1)
            # zero f-padding tile so the DoubleRow pairs contract over KFP
            nc.gpsimd.memset(ah_sb[:, KF, :], 0.0)
            nc.gpsimd.memset(al_sb[:, KF, :], 0.0)

            # x loads: two strided pieces per tensor (chunk0 first so phase A
            # can start, then the remainder), hi before lo (the x_lo passes
            # come last in each accumulation group).  Emitted inside the
            # f-loop so the startup-critical w13 f=0 tile wins the DMA queue.
            cb0 = min(CFG["x_chunk0"], C)
            xpieces = [[(xh_sb, xhd, 0, cb0), (xl_sb, xld, 0, cb0)]]
            if cb0 < C:
                xpieces.append([(xh_sb, xhd, cb0, C - cb0),
                                (xl_sb, xld, cb0, C - cb0)])

            def emit_x(pieces):
                for xsb, xdr, c0, cw in pieces:
                    nc.sync.dma_start(
                        xsb[:, :, c0 : c0 + cw],
                        xdr.rearrange("p (k c) -> p k c", k=KH)[:, :, c0 : c0 + cw],
                    )

            wtiles = {}

            def load_w13(f):
                wt = pool.tile([P, 4, KH, P], F8_DT, tag="w13",
                               bufs=CFG["w13_bufs"])
                nc.sync.dma_start(
                    wt[:].rearrange("p t k j -> p (t k j)"),
                    w13d[:, f * 4 * KH * P : (f + 1) * 4 * KH * P],
                )
                wtiles[f] = wt

            # startup order: w13 f=0 first (first matmul operand), then the
            # x pieces interleaved with the next w13 tile.  All x DMAs must
            # be emitted before any matmul that reads them (the tile
            # framework tracks dependencies in emission order).
            load_w13(0)
            emit_x(xpieces[0])
            load_w13(1)
            for pieces in xpieces[1:]:
                emit_x(pieces)

            # ---- phase A: act[f, c] = silu(x@w1.T) * (x@w3.T), [F, C] layout
            # fp8 DoubleRow, 3 residual passes per psum accumulation group
            # (first compute chunk aligned with the first x piece)
            chunks = _chunks(C, first=CFG["x_chunk0"])
            for f in range(KF):
                if f not in wtiles:
                    load_w13(f)
                wf = wtiles.pop(f)
                if f == CFG["w2_defer_f"]:
                    # defer the (large, phase-B-only) w2 load past startup
                    _split_dma(nc.sync,
                               w2h_sb[:].rearrange("p k h -> p (k h)"),
                               w2hd[:], CFG["w2_split"])
                    _split_dma(nc.sync,
                               w2l_sb[:].rearrange("p k h -> p (k h)"),
                               w2ld[:], CFG["w2_split"])
                for ci, (c0, cb) in enumerate(chunks):
                    ps1 = psum.tile([P, cb], FP32, tag="ps1", bufs=CFG["ps1_bufs"])
                    ps3 = psum.tile([P, cb], FP32, tag="ps3", bufs=CFG["ps1_bufs"])
                    # interleaved psum groups: the four x_hi passes run first
                    # so the startup x_lo transfer has 32 instructions of slack
                    passes = [(ps1, 0, xh_sb, "s"), (ps1, 1, xh_sb, None),
                              (ps3, 2, xh_sb, "s"), (ps3, 3, xh_sb, None),
                              (ps1, 0, xl_sb, "e"), (ps3, 2, xl_sb, "e")]
                    for ps, t, xt, flag in passes:
                        for kk in range(0, KH, 2):
                            nc.tensor.matmul(
                                ps[:],
                                lhsT=wf[:, t, kk : kk + 2, :],
                                rhs=xt[:, kk : kk + 2, c0 : c0 + cb],
                                start=(flag == "s" and kk == 0),
                                stop=(flag == "e" and kk == KH - 2),
                                perf_mode=DR,
                            )
                    tmp = pool.tile([P, cb], BF16_DT, tag="silu", bufs=CFG["silu_bufs"])
                    nc.scalar.activation(
                        tmp[:], ps1[:], mybir.ActivationFunctionType.Silu,
                        scale=DEQ,
                    )
                    h3s = pool.tile([P, cb], BF16_DT, tag="h3s",
                                    bufs=CFG["silu_bufs"])
                    nc.scalar.activation(
                        h3s[:], ps3[:], mybir.ActivationFunctionType.Copy,
                        scale=DEQ * SA,
                    )
                    af = pool.tile([P, cb], BF16_DT, tag="actf",
                                   bufs=CFG["silu_bufs"])
                    nc.vector.tensor_mul(af[:], tmp[:], h3s[:])
                    ah = ah_sb[:, f, c0 : c0 + cb]
                    nc.gpsimd.tensor_copy(ah, af[:])
                    nc.vector.tensor_sub(
                        al_sb[:, f, c0 : c0 + cb], af[:], ah
                    )

            # ---- phase B: y[c, h] = act.T @ w2, fp8 DoubleRow 3-pass
            # (act carries 2^3, w2 carries 2^10; 2^-13 is folded into the
            # host gates / the shared-path epilogue scale)
            for ct in range(-(-C // P)):
                tp = min(P, C - ct * P)   # partial final token-tile
                o = pool.tile([P, HB * 512], out_dt, tag="o", bufs=CFG["o_bufs"])
                for hb in range(HB):
                    ps2 = psum.tile([P, 512], FP32, tag="ps2", bufs=CFG["ps2_bufs"])
                    n = KFP // 2 * 3
                    i = 0
                    for acts, w2s in ((ah_sb, w2h_sb), (al_sb, w2h_sb),
                                      (ah_sb, w2l_sb)):
                        for kk in range(0, KFP, 2):
                            nc.tensor.matmul(
                                ps2[:tp],
                                lhsT=acts[:, kk : kk + 2, ct * P : ct * P + tp],
                                rhs=w2s[:, kk : kk + 2,
                                        hb * 512 : (hb + 1) * 512],
                                start=(i == 0),
                                stop=(i == n - 1),
                                perf_mode=DR,
                            )
                            i += 1
                    if g_sb is not None:
                        nc.vector.tensor_scalar_mul(
                            o[:tp, hb * 512 : (hb + 1) * 512], ps2[:tp],
                            g_sb[:tp, ct : ct + 1]
                        )
                    else:
                        nc.vector.tensor_scalar_mul(
                            o[:tp, hb * 512 : (hb + 1) * 512], ps2[:tp], DEQ2
                        )
                        # shared problem is the kernel tail: store per-hb so
                        # the final DMA isn't gated on all four copies
                        nc.sync.dma_start(
                            yd[ct * P : ct * P + tp,
                               hb * 512 : (hb + 1) * 512],
                            o[:tp, hb * 512 : (hb + 1) * 512],
                        )
                if g_sb is not None:
                    # one store per token-tile covering all HB blocks
                    nc.sync.dma_start(yd[ct * P : ct * P + tp, :], o[:tp, :])

        d = {k: v.ap() for k, v in dram.items()}
        problem("r", d["xrh"], d["xrl"], d["w13"], d["w2h"], d["w2l"],
                yr.ap(), C_r, gd=d["gr"])
        problem("s", d["xsh"], d["xsl"], d["s13"], d["s2h"], d["s2l"],
                ys.ap(), C_s)

    return nc


_cache = {}


def _get_nc(C_r, C_s):
    key = (C_r, C_s, tuple(sorted(CFG.items())))
    if key not in _cache:
        nc = bacc.Bacc("TRN2", target_bir_lowering=False, debug=False,
                       num_devices=NCORES)
        _build(nc, C_r, C_s)
        nc.compile()
        _cache[key] = nc
    return _cache[key]


def _fp8_split(a, scale):
    """v -> (hi, lo) e4m3 pair with v*scale ~= hi + lo."""
    s = np.asarray(a, np.float32) * scale
    np.clip(s, -240.0, 240.0, out=s)
    hi = s.astype(F8)
    lo = s - hi.astype(np.float32)
    np.clip(lo, -240.0, 240.0, out=lo)
    return hi, lo.astype(F8)


def _tile_w13(w):
    """[F, H] -> [128, KF*KH*128], (f, kk, j) column order (any dtype)."""
    return np.ascontiguousarray(
        w.reshape(KF, P, KH, P).transpose(3, 0, 2, 1)
    ).reshape(P, KF * KH * P)


def _tile_w13_fp8(w):
    hi, lo = _fp8_split(w, SW)
    return _tile_w13(hi), _tile_w13(lo)


def _cat_w13(w1, w3):
    """[P, KF*4*KH*P] fp8 with per-f blocks [w1h | w1l | w3h | w3l]."""
    w1h, w1l = _tile_w13_fp8(w1)
    w3h, w3l = _tile_w13_fp8(w3)
    khp = KH * P
    parts = np.stack(
        [a.reshape(P, KF, khp) for a in (w1h, w1l, w3h, w3l)], axis=2
    )
    return np.ascontiguousarray(parts).reshape(P, KF * 4 * khp)


def _tile_w2_fp8(w):
    """[H, F] fp32 -> ([128, KFP*H] hi, lo) fp8, (f, h) order, zero-padded
    to KFP f-tiles."""
    hi, lo = _fp8_split(w, SW2)
    out = []
    for a in (hi, lo):
        t = np.zeros((P, KFP, H), F8)
        t[:, :KF, :] = np.ascontiguousarray(a).reshape(H, KF, P).transpose(2, 1, 0)
        out.append(t.reshape(P, KFP * H))
    return out


def _tile_x(x):
    """[C, H] -> [128, KH*C], (kk, c) column order (any dtype)."""
    C = x.shape[0]
    return np.ascontiguousarray(x.reshape(C, KH, P).transpose(2, 1, 0)).reshape(
        P, KH * C
    )


def _tile_x_fp8(x):
    hi, lo = _fp8_split(x, SX)
    return _tile_x(hi), _tile_x(lo)


def _pad_rows(x, n):
    if x.shape[0] == n:
        return x
    out = np.zeros((n, x.shape[1]), x.dtype)
    out[: x.shape[0]] = x
    return out


def kernel(hidden_states, gate_w, bias, ws1, ws2, ws3, we1, we2, we3):
    orig_shape = hidden_states.shape
    x = np.ascontiguousarray(
        np.asarray(hidden_states, np.float32).reshape(-1, orig_shape[-1])
    )
    T = x.shape[0]
    gate_w = np.asarray(gate_w, np.float32)
    bias = np.asarray(bias, np.float32)
    we1 = np.asarray(we1, np.float32)
    we2 = np.asarray(we2, np.float32)
    we3 = np.asarray(we3, np.float32)
    assert gate_w.shape[0] == E and we1.shape[0] == E and x.shape[1] == H

    # ---- host router (fp32, matches the reference's selection math)
    logits = x @ gate_w.T                                 # [T, E]
    scores = np.where(
        logits >= 0,
        1.0 / (1.0 + np.exp(-np.abs(logits))),
        1.0 - 1.0 / (1.0 + np.exp(-np.abs(logits))),
    ).astype(np.float32)
    routing = scores + bias[None, :]
    topk = np.argsort(-routing, axis=1, kind="stable")[:, :TOPK]  # [T, K]
    sel = np.take_along_axis(scores, topk, axis=1)
    gates = sel / sel.sum(axis=1, keepdims=True)          # [T, K]

    idx_e = []      # token ids routed to expert e
    gate_e = []     # matching combine weights
    for e in range(E):
        mask = topk == e                      # [T, K], at most one True per row
        rows = np.nonzero(mask.any(axis=1))[0]
        idx_e.append(rows)
        gate_e.append(gates[mask].astype(np.float32))  # row-major -> rows order

    max_n = max(len(r) for r in idx_e)
    # multiple of 16: DoubleRow stationary APs stride by C bytes and the
    # PE requires that stride to be 16-byte aligned
    C_r = max(64, -(-max_n // 16) * 16)
    C_s = max(64, -(-T // (NCORES * 64)) * 64)  # shared tokens per core

    nc = _get_nc(C_r, C_s)

    # ---- build per-core input maps
    s2h, s2l = _tile_w2_fp8(ws2)
    shared_w = {
        "s13": _cat_w13(ws1, ws3),
        "s2h": s2h, "s2l": s2l,
    }
    in_maps = []
    for e in range(E):
        rows = idx_e[e]
        xg = np.zeros((C_r, H), np.float32)
        xg[: len(rows)] = x[rows]
        ctiles = -(-C_r // P)
        g = np.zeros((ctiles * P,), np.float32)
        g[: len(rows)] = gate_e[e] * DEQ2
        xrh, xrl = _tile_x_fp8(xg)
        xsh, xsl = _tile_x_fp8(_pad_rows(x[e * C_s : (e + 1) * C_s], C_s))
        w2h, w2l = _tile_w2_fp8(we2[e])
        m = {
            "xrh": xrh, "xrl": xrl,
            "gr": np.ascontiguousarray(g.reshape(ctiles, P).T),
            "w13": _cat_w13(we1[e], we3[e]),
            "w2h": w2h, "w2l": w2l,
            "xsh": xsh, "xsl": xsl,
        }
        m.update(shared_w)
        in_maps.append(m)

    global _active_build_key
    _active_build_key = _build_key(C_r, C_s)
    try:
        res = run_bass_kernel_spmd(nc, in_maps, list(range(NCORES))).results
    finally:
        _active_build_key = None

    # ---- host combine
    out = np.zeros((T, H), np.float32)
    for e in range(E):
        rows = idx_e[e]
        out[rows] += np.asarray(res[e]["yr"][: len(rows)], np.float32)
        lo = e * C_s
        hi = min(T, (e + 1) * C_s)
        if lo < hi:
            out[lo:hi] += np.asarray(res[e]["ys"][: hi - lo], np.float32)
    return out.reshape(orig_shape).astype(np.float32)
